# revision 19
# baseline (speedup 1.0000x reference)
"""DSS layer (S4-style diagonal state space) Trainium2 kernel, v5.

Full inputs:  u [8,128,4096], Lambda_re/im [128,64], W_ri [128,64,2],
              D [128], log_step [128]  ->  y [8,128,4096]

Sharding: H split 8 ways (16 channels/core), all B on every core.

Algorithm (chunked semiseparable scan, C=256, T=16 chunks):
  With z = exp(step*Lambda) (|z|<1), the DSS kernel is
      K[l] = Re sum_n c_n z_n^l,   c = (W/Lambda)*conj(s)/(|s|^2+eps),
      s = (1-z^L)/(1-z)
  and y = causal_conv(K, u) + D*u. Per (b,h) the conv splits into
    intra-chunk:  dense Toeplitz matmul with kshift[j,i] = K[i-j]
                  (+ D on the diagonal), causal-masked
    inter-chunk:  states s[J] = sum_j z^{C-1-j} u[J*C+j]  (matmul),
                  scan h[I] = z^C h[I-1] + s[I-1]          (vector),
                  y_inter = Re sum_n c z^{i+1} h_n[I]      (matmul).

v5: radix-4 scan (pairs -> quads -> 3-step chain -> two fill levels,
35 contiguous vector ops); w tables expanded on-device; DMA priority
order (u/zft first, ksh behind them on the same queue, e2id issue
data-gated behind stage 1); small fp32 matmuls chained to scan outputs
keep the PE p-state up through the scan window; bf16 matmuls + output.
"""
import numpy as np
import ml_dtypes

import concourse.bass as bass
import concourse.bacc as bacc
import concourse.tile as tile
from concourse import mybir
from concourse.bass_utils import run_bass_kernel_spmd

F32 = mybir.dt.float32
BF16 = mybir.dt.bfloat16
BF16_NP = ml_dtypes.bfloat16

B, H, L, N = 8, 128, 4096, 64
NCORES = 8
HL = H // NCORES            # 16 channels per core
C = 256                     # chunk length
T = L // C                  # 16 chunks
TK = T // 2                 # 8 chunk pairs
NQ = HL // 2                # 8 h-pairs
NK = TK - 1                 # 7 pair states feeding the chain
EPS = 1e-7

# e2id col offsets: e2a | e2b | ident
E2A_O, E2B_O, ID_O = 0, NQ * C, 2 * NQ * C
E2ID_W = ID_O + 128
# wf col offsets (fp32): w2p1 | w2p2 | w4p1 | w4p2 | w81 | w82
W2P1_O, W2P2_O, W41_O, W42_O, W81_O, W82_O = 0, 128, 256, 384, 512, 640
WF_W = 768

_CACHE = {}


def _build():
    """Build the SPMD Bass program (identical on all cores)."""
    nc = bacc.Bacc(trn_type="TRN2", target_bir_lowering=False)

    # ---------------- DRAM I/O (per core) ----------------
    # ut_k[p, h*128 + (J%2)*64 + (J//2)*8 + b] = u[b, h, J*256 + k*128 + p]
    ut0_d = nc.dram_tensor("ut0", [128, T * 128], BF16, kind="ExternalInput")
    ut1_d = nc.dram_tensor("ut1", [128, T * 128], BF16, kind="ExternalInput")
    # zft_k[p, h*128+r*64+n] = Re/Im(z^{(255|127)-p})
    zft0_d = nc.dram_tensor("zft0", [128, HL * 128], BF16, kind="ExternalInput")
    zft1_d = nc.dram_tensor("zft1", [128, HL * 128], BF16, kind="ExternalInput")
    # ksh[p, h*C+m] = K[h,m-p] (m>p), K[h,0]+D[h] (m=p), 0 (m<p)
    ksh_d = nc.dram_tensor("ksh", [128, HL * C], BF16, kind="ExternalInput")
    # e2a[par*64+n, q*C+i] = Re(c z^{i+1}), e2b = -Im(c z^{i+1}); ident
    e2id_d = nc.dram_tensor("e2id", [128, E2ID_W], BF16, kind="ExternalInput")
    # scan tables fp32, packed re|im cols (r, q, b): w^2, w^4, w
    wf_d = nc.dram_tensor("wf", [128, WF_W], F32, kind="ExternalInput")

    # rows = (J%2)*64 + (J//2)*8 + b, cols = h*C+c  (host un-permutes)
    y_d = nc.dram_tensor("y_s", [128, HL * C], BF16, kind="ExternalOutput")

    with tile.TileContext(nc) as tc, \
         tc.tile_pool(name="big", bufs=1) as big, \
         tc.tile_pool(name="work", bufs=2) as work, \
         tc.tile_pool(name="workbig", bufs=1) as workbig, \
         tc.tile_pool(name="ps1", bufs=2, space="PSUM") as ps1, \
         tc.tile_pool(name="psy", bufs=6, space="PSUM") as psy:

        # ---- DMA issue: u/zft gate stage 1, ksh queued behind u ----
        zft0 = big.tile([128, HL * 128], BF16, tag="zft0")
        zft1 = big.tile([128, HL * 128], BF16, tag="zft1")
        ut0 = big.tile([128, T * 128], BF16, tag="ut0")
        ut1 = big.tile([128, T * 128], BF16, tag="ut1")
        ksh = big.tile([128, HL * C], BF16, tag="ksh")
        e2id = big.tile([128, E2ID_W], BF16, tag="e2id")
        wf = big.tile([128, WF_W], F32, tag="wf")
        nc.scalar.dma_start(out=zft0, in_=zft0_d[:])
        nc.sync.dma_start(out=ut0, in_=ut0_d[:])
        nc.scalar.dma_start(out=zft1, in_=zft1_d[:])
        nc.scalar.dma_start(out=wf, in_=wf_d[:])
        nc.sync.dma_start(out=ut1, in_=ut1_d[:])
        nc.sync.dma_start(out=ksh, in_=ksh_d[:])

        e2a = e2id[:, E2A_O:E2A_O + NQ * C]
        e2b = e2id[:, E2B_O:E2B_O + NQ * C]
        ident = e2id[:, ID_O:ID_O + 128]

        TT = nc.vector.tensor_tensor
        VC = nc.vector.tensor_copy
        MUL = mybir.AluOpType.mult
        ADD = mybir.AluOpType.add
        SUB = mybir.AluOpType.subtract
        uT = [ut0, ut1]
        zft = [zft0, zft1]

        # packed state tiles: cols (J%2, J//2, r, q, b), r = re|im
        SSP = big.tile([128, T * 128], F32, tag="ssp")
        HSP = big.tile([128, T * 128], F32, tag="hsp")
        nc.vector.memset(HSP[:, 0:128], 0.0)

        # expand w tables over chunk-pair groups (vector, idle window)
        w2a = big.tile([128, 512], F32, tag="w2a")
        w2b = big.tile([128, 512], F32, tag="w2b")
        w8a = big.tile([128, TK * 128], F32, tag="w8a")
        w8b = big.tile([128, TK * 128], F32, tag="w8b")
        for dst, off, reps, eng in (
                (w2a, W2P1_O, 2, nc.gpsimd), (w2b, W2P2_O, 2, nc.gpsimd),
                (w8a, W81_O, 3, nc.vector), (w8b, W82_O, 3, nc.vector)):
            eng.tensor_copy(dst[:, 0:128], wf[:, off:off + 128])
            for i in range(reps):
                n = 128 << i
                eng.tensor_copy(dst[:, n:2 * n], dst[:, 0:n])

        # ---------------- stage 1: chunk states ----------------------------
        SSPv = SSP.rearrange("p (p2 k r q b) -> p p2 k r q b",
                             p2=2, k=TK, r=2, q=NQ)
        for h in range(HL):
            q, par = h // 2, h % 2
            ps_s = ps1.tile([128, 128], F32, tag='ps1t')
            for k in range(2):
                nc.tensor.matmul(ps_s, zft[k][:, h * 128:(h + 1) * 128],
                                 uT[k][:, h * 128:(h + 1) * 128],
                                 start=(k == 0), stop=(k == 1))
            sl = slice(par * 64, (par + 1) * 64)
            srcv = ps_s.rearrange("p (p2 k b) -> p p2 k b", p2=2, k=TK)
            VC(SSPv[sl, :, :, 0, q, :], srcv[0:64])
            nc.scalar.copy(SSPv[sl, :, :, 1, q, :], srcv[64:128])
            if h == 0:
                # WAW dep: write into e2id tile from SSP so the e2id DMA
                # cannot start before stage 1 is running (bandwidth gate)
                nc.gpsimd.tensor_copy(e2id[0:1, 0:4], SSP[0:1, 0:4])
                nc.gpsimd.dma_start(out=e2id, in_=e2id_d[:])

        # ---------------- stage 3a: intra-chunk (scan-independent) ---------
        ybf = big.tile([128, HL * C], F32, tag="ybf")
        for h in range(HL):
            ps_y = psy.tile([128, C], F32)
            nc.tensor.matmul(ps_y, ut0[:, h * 128:(h + 1) * 128],
                             ksh[:, h * C:h * C + C], start=True, stop=False)
            nc.tensor.matmul(ps_y[:, 128:C], ut1[:, h * 128:(h + 1) * 128],
                             ksh[:, h * C:h * C + 128], start=False, stop=True)
            nc.scalar.copy(ybf[:, h * C:(h + 1) * C], ps_y)

        # ---------------- stage 2: radix-4 scan (vector, fp32) -------------
        # warm-keeper: tiny fp32 matmul chained to a freshly written tile
        def warm(src128):
            psd = ps1.tile([128, 128], F32, tag='ps1t')
            nc.tensor.matmul(psd, wf[:, 0:128], src128,
                             start=True, stop=True)

        def cmul5(dst, dstv, wa, wb, src, add_t, width):
            """dst = w (.) src + add_t; dstv = [p, k, r, 64] view of dst."""
            m1 = workbig.tile([128, width], F32, tag=f"m1_{width}")
            TT(out=m1, in0=wa, in1=src, op=MUL)
            m2 = workbig.tile([128, width], F32, tag=f"m2_{width}")
            TT(out=m2, in0=wb, in1=src, op=MUL)
            m1v = m1.rearrange("p (k r x) -> p k r x", r=2, x=64)
            m2v = m2.rearrange("p (k r x) -> p k r x", r=2, x=64)
            TT(out=dstv[:, :, 0], in0=m1v[:, :, 0], in1=m1v[:, :, 1], op=SUB)
            TT(out=dstv[:, :, 1], in0=m2v[:, :, 0], in1=m2v[:, :, 1], op=ADD)
            TT(out=dst, in0=dst, in1=add_t, op=ADD)

        def g3(t, nk):          # [p, nk, 128] view of contiguous 128-groups
            return t.rearrange("p (k g) -> p k g", k=nk) if nk > 1 else t

        def s3(t, nk, g0=0):    # [p, nk, 128] view at stride 256 from g0
            a0 = t[:, g0 * 128:g0 * 128 + 128]
            return bass.AP(tensor=a0.tensor, offset=a0.offset,
                           ap=[list(a0.ap[0]), [256, nk], [1, 128]])

        def cmul(dst, wa, wb, src, add_t, nk):
            """dst = w (.) src + add_t over nk packed (r,q,b) 128-groups.

            dst must be a contiguous [128, nk*128] region; wa/wb contiguous
            weight regions; src/add_t any [p, nk, 128]-shaped views.
            """
            m1 = workbig.tile([128, nk * 128], F32, tag=f"m1_{nk}")
            TT(out=g3(m1, nk), in0=g3(wa, nk), in1=src, op=MUL)
            m2 = workbig.tile([128, nk * 128], F32, tag=f"m2_{nk}")
            TT(out=g3(m2, nk), in0=g3(wb, nk), in1=src, op=MUL)
            m1v = m1.rearrange("p (k r x) -> p k r x", r=2, x=64)
            m2v = m2.rearrange("p (k r x) -> p k r x", r=2, x=64)
            dv = dst.rearrange("p (k r x) -> p k r x", r=2, x=64)
            TT(out=dv[:, :, 0], in0=m1v[:, :, 0], in1=m1v[:, :, 1], op=SUB)
            TT(out=dv[:, :, 1], in0=m2v[:, :, 0], in1=m2v[:, :, 1], op=ADD)
            TT(out=g3(dst, nk), in0=g3(dst, nk), in1=add_t, op=ADD)

        SSPk = SSP.rearrange("p (p2 k g) -> p p2 k g", p2=2, k=TK)
        # G[k] = w (.) SS[2k] + SS[2k+1], k=0..6
        GP = big.tile([128, NK * 128], F32, tag="gp")
        cmul(GP, w8a[:, 0:NK * 128], w8b[:, 0:NK * 128],
             SSPk[:, 0, 0:NK, :], SSPk[:, 1, 0:NK, :], NK)
        warm(GP[:, 0:128])

        # G2[k] = w^2 (.) G[2k] + G[2k+1], k=0..2
        G2 = big.tile([128, 3 * 128], F32, tag="g2")
        cmul(G2, w2a[:, 0:384], w2b[:, 0:384],
             s3(GP, 3), s3(GP, 3, g0=1), 3)
        warm(G2[:, 0:128])

        # chain over h[4k]: 3 steps, h[4k+4] = w^4 (.) h[4k] + G2[k]
        for j in range(3):
            prev = HSP[:, (2 * j) * 128:(2 * j + 1) * 128]
            nxt = HSP[:, (2 * j + 2) * 128:(2 * j + 3) * 128]
            m1 = work.tile([128, 128], F32, tag="c4m1")
            TT(out=m1, in0=wf[:, W41_O:W41_O + 128], in1=prev, op=MUL)
            m2 = work.tile([128, 128], F32, tag="c4m2")
            TT(out=m2, in0=wf[:, W42_O:W42_O + 128], in1=prev, op=MUL)
            TT(out=nxt[:, 0:64], in0=m1[:, 0:64], in1=m1[:, 64:128], op=SUB)
            TT(out=nxt[:, 64:128], in0=m2[:, 0:64], in1=m2[:, 64:128], op=ADD)
            TT(out=nxt, in0=nxt, in1=G2[:, j * 128:(j + 1) * 128], op=ADD)
            warm(nxt)

        # fill h[4k+2] = w^2 (.) h[4k] + G[2k], k=0..3: tiles {1,3,5,7} of
        # the even half from tiles {0,2,4,6} and G tiles {0,2,4,6}
        f2 = workbig.tile([128, 512], F32, tag="f2")
        cmul(f2, w2a, w2b, s3(HSP, 4), s3(GP, 4), 4)
        HOt = HSP.rearrange("p (half k2 two g) -> p half k2 two g",
                            half=2, k2=4, two=2)[:, 0, :, 1, :]
        VC(HOt, g3(f2, 4))
        warm(HSP[:, 7 * 128:8 * 128])

        # odd fill: h[2k+1] = w (.) h[2k] + SS[2k], all k at once
        HE, HO = HSP[:, 0:1024], HSP[:, 1024:2048]
        cmul(HO, w8a, w8b, g3(HE, TK), SSPk[:, 0], TK)
        warm(HO[:, 0:128])

        # repack (p2, k, r, q, b) -> r-split (q, p2, k, b) bf16 for lhsT
        HS2r = big.tile([128, NQ * 128], BF16, tag="hs2r")
        HS2i = big.tile([128, NQ * 128], BF16, tag="hs2i")
        HSPq = HSP.rearrange("p (p2 k r q b) -> p p2 r q k b",
                             p2=2, k=TK, r=2, q=NQ)
        for p2 in range(2):
            for r, dst in ((0, HS2r), (1, HS2i)):
                dv = dst.rearrange("p (q p2 k b) -> p p2 q k b",
                                   q=NQ, p2=2, k=TK)[:, p2]
                VC(dv, HSPq[:, p2, r])

        # ---------------- stage 3b: inter-chunk + intra add + output -------
        y_out = big.tile([128, HL * C], BF16, tag="yout")
        for h in range(HL):
            q, par = h // 2, h % 2
            sl = slice(par * 64, (par + 1) * 64)
            cs = slice(q * 128, (q + 1) * 128)
            ps_y = psy.tile([128, C], F32)
            nc.tensor.matmul(ps_y, HS2r[sl, cs],
                             e2a[sl, q * C:(q + 1) * C],
                             start=True, stop=False)
            nc.tensor.matmul(ps_y, HS2i[sl, cs],
                             e2b[sl, q * C:(q + 1) * C],
                             start=False, stop=True)
            TT(out=y_out[:, h * C:(h + 1) * C],
               in0=ybf[:, h * C:(h + 1) * C], in1=ps_y, op=ADD)
            if h % 4 == 3:
                nc.sync.dma_start(out=y_d[:, (h - 3) * C:(h + 1) * C],
                                  in_=y_out[:, (h - 3) * C:(h + 1) * C])

    nc.compile()
    return nc


def _host_prep(u, Lambda_re, Lambda_im, W_ri, D, log_step):
    """Per-core input dicts; all tables in float64 then cast."""
    u = np.asarray(u, np.float32)
    Lr = np.asarray(Lambda_re, np.float64)
    Li = np.asarray(Lambda_im, np.float64)
    W = np.asarray(W_ri, np.float64)
    Dv = np.asarray(D, np.float64)
    ls = np.asarray(log_step, np.float64)

    step = np.exp(ls)                                    # [H]
    a = step[:, None] * Lr                               # [H,N]
    th = step[:, None] * Li
    lam = a + 1j * th                                    # log z
    z = np.exp(lam)
    w = z ** C
    zL = z ** L
    s = (1.0 - zL) / (1.0 - z)
    Lam = Lr + 1j * Li
    c = (W[..., 0] + 1j * W[..., 1]) / Lam * np.conj(s) / (s * np.conj(s) + EPS)

    e = np.arange(C + 1, dtype=np.float64)
    zp = np.exp(lam[..., None] * e)                      # [H,N,C+1]
    cz = c[..., None] * zp                               # [H,N,C+1]
    K = cz.real.sum(axis=1)                              # [H,C+1]

    # Toeplitz index helpers
    pr = np.arange(128)
    dmat = np.arange(C)[None, :] - pr[:, None]           # [128,C]
    valid = dmat >= 0
    dcl = np.where(valid, dmat, 0)

    idmat = np.eye(128, dtype=np.float64)

    in_maps = []
    for core in range(NCORES):
        hs = slice(core * HL, (core + 1) * HL)

        # u cols (h, J%2, J//2, b): [B,HL,kk,p2,k,128] -> [k][p][h][p2][kk][b]
        uu = np.asarray(u[:, hs]).reshape(B, HL, TK, 2, 2, 128)
        uu_t = np.ascontiguousarray(
            uu.transpose(4, 5, 1, 3, 2, 0)).reshape(2, 128, T * 128)

        # kshift Toeplitz blocks
        Kh = K[hs]                                       # [HL,C+1]
        M = Kh[:, dcl] * valid[None]                     # [HL,128,C]
        M[:, pr, pr] += Dv[hs, None]
        ksh = np.ascontiguousarray(M.transpose(1, 0, 2)).reshape(128, HL * C)

        # stage-1 weights
        zz = zp[hs]                                      # [HL,N,C+1]
        P0 = zz[:, :, 255 - pr]                          # [HL,N,128]
        P1 = zz[:, :, 127 - pr]
        zft0 = np.stack([P0.real, P0.imag], axis=1)      # [HL,2,N,128]
        zft1 = np.stack([P1.real, P1.imag], axis=1)
        zft0 = np.ascontiguousarray(
            zft0.transpose(3, 0, 1, 2)).reshape(128, HL * 128)
        zft1 = np.ascontiguousarray(
            zft1.transpose(3, 0, 1, 2)).reshape(128, HL * 128)

        # stage-3 tables, j = i+1 (cols q*C+i): [q,par,n,j]->rows (par,n)
        czs = cz[hs, :, 1:].reshape(NQ, 2, N, C)
        czt = np.ascontiguousarray(
            czs.transpose(1, 2, 0, 3)).reshape(128, NQ * C)
        e2id = np.concatenate([czt.real, -czt.imag, idmat], axis=1)

        def wpack(x):  # [HL,N] -> rows (par,n), cols (q,b)
            return np.ascontiguousarray(
                np.asarray(x).reshape(HL // 2, 2, N).transpose(1, 2, 0)
            ).reshape(128, HL // 2, 1).repeat(8, axis=2).reshape(128, 64)

        w1, w2, w4 = w[hs], w[hs] ** 2, w[hs] ** 4
        wf = np.concatenate([
            wpack(w2.real), wpack(w2.imag), wpack(w2.imag), wpack(w2.real),
            wpack(w4.real), wpack(w4.imag), wpack(w4.imag), wpack(w4.real),
            wpack(w1.real), wpack(w1.imag), wpack(w1.imag), wpack(w1.real)],
            axis=1)

        m = {
            "ut0": uu_t[0].astype(BF16_NP),
            "ut1": uu_t[1].astype(BF16_NP),
            "zft0": zft0.astype(BF16_NP),
            "zft1": zft1.astype(BF16_NP),
            "ksh": ksh.astype(BF16_NP),
            "e2id": e2id.astype(BF16_NP),
            "wf": wf.astype(np.float32),
        }
        in_maps.append(m)
    return in_maps


def _run(inputs, trace=False):
    if "nc" not in _CACHE:
        _CACHE["nc"] = _build()
    nc = _CACHE["nc"]
    in_maps = _host_prep(**inputs)
    res = run_bass_kernel_spmd(nc, in_maps, list(range(NCORES)), trace=trace)
    parts = []
    for core in range(NCORES):
        # rows (J%2, J//2, b); (kk, p2) reshape restores J = 2*kk + J%2
        ys = res.results[core]["y_s"].astype(np.float32)
        ys = ys.reshape(2, TK, B, HL, C).transpose(2, 3, 1, 0, 4)
        parts.append(ys.reshape(B, HL, L))
    y = np.concatenate(parts, axis=1)                    # [B, H, L]
    return np.ascontiguousarray(y.astype(np.float32)), res


def kernel(**inputs) -> np.ndarray:
    y, _ = _run(inputs, trace=False)
    return y


def kernel_traced(**inputs):
    y, res = _run(inputs, trace=True)
    return y, res


# revision 20
# speedup vs baseline: 1.0516x; 1.0516x over previous
"""DSS layer (S4-style diagonal state space) Trainium2 kernel, v5.

Full inputs:  u [8,128,4096], Lambda_re/im [128,64], W_ri [128,64,2],
              D [128], log_step [128]  ->  y [8,128,4096]

Sharding: H split 8 ways (16 channels/core), all B on every core.

Algorithm (chunked semiseparable scan, C=256, T=16 chunks):
  With z = exp(step*Lambda) (|z|<1), the DSS kernel is
      K[l] = Re sum_n c_n z_n^l,   c = (W/Lambda)*conj(s)/(|s|^2+eps),
      s = (1-z^L)/(1-z)
  and y = causal_conv(K, u) + D*u. Per (b,h) the conv splits into
    intra-chunk:  dense Toeplitz matmul with kshift[j,i] = K[i-j]
                  (+ D on the diagonal), causal-masked
    inter-chunk:  states s[J] = sum_j z^{C-1-j} u[J*C+j]  (matmul),
                  scan h[I] = z^C h[I-1] + s[I-1]          (vector),
                  y_inter = Re sum_n c z^{i+1} h_n[I]      (matmul).

v5: radix-4 scan (pairs -> quads -> 3-step chain -> two fill levels,
35 contiguous vector ops); w tables expanded on-device; DMA priority
order (u/zft first, ksh behind them on the same queue, e2id issue
data-gated behind stage 1); small fp32 matmuls chained to scan outputs
keep the PE p-state up through the scan window; bf16 matmuls + output.
"""
import numpy as np
import ml_dtypes

import concourse.bass as bass
import concourse.bacc as bacc
import concourse.tile as tile
from concourse import mybir
from concourse.bass_utils import run_bass_kernel_spmd

F32 = mybir.dt.float32
BF16 = mybir.dt.bfloat16
BF16_NP = ml_dtypes.bfloat16

B, H, L, N = 8, 128, 4096, 64
NCORES = 8
HL = H // NCORES            # 16 channels per core
C = 256                     # chunk length
T = L // C                  # 16 chunks
TK = T // 2                 # 8 chunk pairs
NQ = HL // 2                # 8 h-pairs
NK = TK - 1                 # 7 pair states feeding the chain
EPS = 1e-7

# e2id col offsets: e2a | e2b | ident
E2A_O, E2B_O, ID_O = 0, NQ * C, 2 * NQ * C
E2ID_W = ID_O + 128
# wf col offsets (fp32): w2p1 | w2p2 | w4p1 | w4p2 | w81 | w82
W2P1_O, W2P2_O, W41_O, W42_O, W81_O, W82_O = 0, 128, 256, 384, 512, 640
WF_W = 768

_CACHE = {}


def _build():
    """Build the SPMD Bass program (identical on all cores)."""
    nc = bacc.Bacc(trn_type="TRN2", target_bir_lowering=False)

    # ---------------- DRAM I/O (per core) ----------------
    # ut_k[p, h*128 + (J%2)*64 + (J//2)*8 + b] = u[b, h, J*256 + k*128 + p]
    ut0_d = nc.dram_tensor("ut0", [128, T * 128], BF16, kind="ExternalInput")
    ut1_d = nc.dram_tensor("ut1", [128, T * 128], BF16, kind="ExternalInput")
    # zft_k[p, h*128+r*64+n] = Re/Im(z^{(255|127)-p})
    zft0_d = nc.dram_tensor("zft0", [128, HL * 128], BF16, kind="ExternalInput")
    zft1_d = nc.dram_tensor("zft1", [128, HL * 128], BF16, kind="ExternalInput")
    # ksh[p, h*C+m] = K[h,m-p] (m>p), K[h,0]+D[h] (m=p), 0 (m<p)
    ksh_d = nc.dram_tensor("ksh", [128, HL * C], BF16, kind="ExternalInput")
    # e2a[par*64+n, q*C+i] = Re(c z^{i+1}), e2b = -Im(c z^{i+1}); ident
    e2id_d = nc.dram_tensor("e2id", [128, E2ID_W], BF16, kind="ExternalInput")
    # scan tables fp32, packed re|im cols (r, q, b): w^2, w^4, w
    wf_d = nc.dram_tensor("wf", [128, WF_W], F32, kind="ExternalInput")

    # rows = (J%2)*64 + (J//2)*8 + b, cols = h*C+c  (host un-permutes)
    y_d = nc.dram_tensor("y_s", [128, HL * C], BF16, kind="ExternalOutput")

    with tile.TileContext(nc) as tc, \
         tc.tile_pool(name="big", bufs=1) as big, \
         tc.tile_pool(name="work", bufs=2) as work, \
         tc.tile_pool(name="workbig", bufs=1) as workbig, \
         tc.tile_pool(name="ps1", bufs=4, space="PSUM") as ps1, \
         tc.tile_pool(name="psy", bufs=4, space="PSUM") as psy:

        # ---- DMA issue: u/zft gate stage 1, ksh queued behind u ----
        zft0 = big.tile([128, HL * 128], BF16, tag="zft0")
        zft1 = big.tile([128, HL * 128], BF16, tag="zft1")
        ut0 = big.tile([128, T * 128], BF16, tag="ut0")
        ut1 = big.tile([128, T * 128], BF16, tag="ut1")
        ksh = big.tile([128, HL * C], BF16, tag="ksh")
        e2id = big.tile([128, E2ID_W], BF16, tag="e2id")
        wf = big.tile([128, WF_W], F32, tag="wf")
        nc.scalar.dma_start(out=zft0, in_=zft0_d[:])
        nc.sync.dma_start(out=ut0, in_=ut0_d[:])
        nc.scalar.dma_start(out=zft1, in_=zft1_d[:])
        nc.scalar.dma_start(out=wf, in_=wf_d[:])
        nc.sync.dma_start(out=ut1, in_=ut1_d[:])
        nc.sync.dma_start(out=ksh, in_=ksh_d[:])

        e2a = e2id[:, E2A_O:E2A_O + NQ * C]
        e2b = e2id[:, E2B_O:E2B_O + NQ * C]
        ident = e2id[:, ID_O:ID_O + 128]

        TT = nc.vector.tensor_tensor
        VC = nc.vector.tensor_copy
        MUL = mybir.AluOpType.mult
        ADD = mybir.AluOpType.add
        SUB = mybir.AluOpType.subtract
        uT = [ut0, ut1]
        zft = [zft0, zft1]

        # packed state tiles: cols (J%2, J//2, r, q, b), r = re|im
        SSP = big.tile([128, T * 128], F32, tag="ssp")
        HSP = big.tile([128, T * 128], F32, tag="hsp")
        nc.vector.memset(HSP[:, 0:128], 0.0)

        # expand w tables over chunk-pair groups (vector, idle window)
        w2a = big.tile([128, 512], F32, tag="w2a")
        w2b = big.tile([128, 512], F32, tag="w2b")
        w8a = big.tile([128, TK * 128], F32, tag="w8a")
        w8b = big.tile([128, TK * 128], F32, tag="w8b")
        for dst, off, reps, eng in (
                (w8a, W81_O, 3, nc.gpsimd), (w8b, W82_O, 3, nc.gpsimd),
                (w2a, W2P1_O, 2, nc.gpsimd), (w2b, W2P2_O, 2, nc.gpsimd)):
            eng.tensor_copy(dst[:, 0:128], wf[:, off:off + 128])
            for i in range(reps):
                n = 128 << i
                eng.tensor_copy(dst[:, n:2 * n], dst[:, 0:n])

        # ---------------- stage 1: chunk states ----------------------------
        SSPv = SSP.rearrange("p (p2 k r q b) -> p p2 k r q b",
                             p2=2, k=TK, r=2, q=NQ)
        for h in range(HL):
            q, par = h // 2, h % 2
            ps_s = ps1.tile([128, 128], F32, tag='ps1t')
            for k in range(2):
                nc.tensor.matmul(ps_s, zft[k][:, h * 128:(h + 1) * 128],
                                 uT[k][:, h * 128:(h + 1) * 128],
                                 start=(k == 0), stop=(k == 1))
            sl = slice(par * 64, (par + 1) * 64)
            srcv = ps_s.rearrange("p (p2 k b) -> p p2 k b", p2=2, k=TK)
            VC(SSPv[sl, :, :, 0, q, :], srcv[0:64])
            VC(SSPv[sl, :, :, 1, q, :], srcv[64:128])
            if h == 0:
                # WAW dep: write into e2id tile from SSP so the e2id DMA
                # cannot start before stage 1 is running (bandwidth gate)
                nc.gpsimd.tensor_copy(e2id[0:1, 0:4], SSP[0:1, 0:4])
                nc.gpsimd.dma_start(out=e2id, in_=e2id_d[:])

        # ---------------- stage 3a: intra-chunk (scan-independent) ---------
        ybf = big.tile([128, HL * C], F32, tag="ybf")
        for h in range(HL):
            ps_y = psy.tile([128, C], F32)
            nc.tensor.matmul(ps_y, ut0[:, h * 128:(h + 1) * 128],
                             ksh[:, h * C:h * C + C], start=True, stop=False)
            nc.tensor.matmul(ps_y[:, 128:C], ut1[:, h * 128:(h + 1) * 128],
                             ksh[:, h * C:h * C + 128], start=False, stop=True)
            nc.scalar.copy(ybf[:, h * C:(h + 1) * C], ps_y)

        # ---------------- stage 2: radix-4 scan (vector, fp32) -------------
        # warm-keeper: tiny fp32 matmul chained to a freshly written tile
        def warm(src128):
            psd = ps1.tile([128, 128], F32, tag='ps1t')
            nc.tensor.matmul(psd, wf[:, 0:128], src128,
                             start=True, stop=True)

        def cmul5(dst, dstv, wa, wb, src, add_t, width):
            """dst = w (.) src + add_t; dstv = [p, k, r, 64] view of dst."""
            m1 = workbig.tile([128, width], F32, tag=f"m1_{width}")
            TT(out=m1, in0=wa, in1=src, op=MUL)
            m2 = workbig.tile([128, width], F32, tag=f"m2_{width}")
            TT(out=m2, in0=wb, in1=src, op=MUL)
            m1v = m1.rearrange("p (k r x) -> p k r x", r=2, x=64)
            m2v = m2.rearrange("p (k r x) -> p k r x", r=2, x=64)
            TT(out=dstv[:, :, 0], in0=m1v[:, :, 0], in1=m1v[:, :, 1], op=SUB)
            TT(out=dstv[:, :, 1], in0=m2v[:, :, 0], in1=m2v[:, :, 1], op=ADD)
            TT(out=dst, in0=dst, in1=add_t, op=ADD)

        def g3(t, nk):          # [p, nk, 128] view of contiguous 128-groups
            return t.rearrange("p (k g) -> p k g", k=nk) if nk > 1 else t

        def s3(t, nk, g0=0):    # [p, nk, 128] view at stride 256 from g0
            a0 = t[:, g0 * 128:g0 * 128 + 128]
            return bass.AP(tensor=a0.tensor, offset=a0.offset,
                           ap=[list(a0.ap[0]), [256, nk], [1, 128]])

        def cmul(dst, wa, wb, src, add_t, nk):
            """dst = w (.) src + add_t over nk packed (r,q,b) 128-groups.

            dst must be a contiguous [128, nk*128] region; wa/wb contiguous
            weight regions; src/add_t any [p, nk, 128]-shaped views.
            """
            m1 = workbig.tile([128, nk * 128], F32, tag=f"m1_{nk}")
            TT(out=g3(m1, nk), in0=g3(wa, nk), in1=src, op=MUL)
            m2 = workbig.tile([128, nk * 128], F32, tag=f"m2_{nk}")
            TT(out=g3(m2, nk), in0=g3(wb, nk), in1=src, op=MUL)
            m1v = m1.rearrange("p (k r x) -> p k r x", r=2, x=64)
            m2v = m2.rearrange("p (k r x) -> p k r x", r=2, x=64)
            dv = dst.rearrange("p (k r x) -> p k r x", r=2, x=64)
            TT(out=dv[:, :, 0], in0=m1v[:, :, 0], in1=m1v[:, :, 1], op=SUB)
            TT(out=dv[:, :, 1], in0=m2v[:, :, 0], in1=m2v[:, :, 1], op=ADD)
            TT(out=g3(dst, nk), in0=g3(dst, nk), in1=add_t, op=ADD)

        SSPk = SSP.rearrange("p (p2 k g) -> p p2 k g", p2=2, k=TK)
        # G[k] = w (.) SS[2k] + SS[2k+1], k=0..6
        GP = big.tile([128, NK * 128], F32, tag="gp")
        cmul(GP, w8a[:, 0:NK * 128], w8b[:, 0:NK * 128],
             SSPk[:, 0, 0:NK, :], SSPk[:, 1, 0:NK, :], NK)
        warm(GP[:, 0:128])

        # G2[k] = w^2 (.) G[2k] + G[2k+1], k=0..2
        G2 = big.tile([128, 3 * 128], F32, tag="g2")
        cmul(G2, w2a[:, 0:384], w2b[:, 0:384],
             s3(GP, 3), s3(GP, 3, g0=1), 3)
        warm(G2[:, 0:128])

        # chain over h[4k]: 3 steps, h[4k+4] = w^4 (.) h[4k] + G2[k]
        for j in range(3):
            prev = HSP[:, (2 * j) * 128:(2 * j + 1) * 128]
            nxt = HSP[:, (2 * j + 2) * 128:(2 * j + 3) * 128]
            m1 = work.tile([128, 128], F32, tag="c4m1")
            TT(out=m1, in0=wf[:, W41_O:W41_O + 128], in1=prev, op=MUL)
            m2 = work.tile([128, 128], F32, tag="c4m2")
            TT(out=m2, in0=wf[:, W42_O:W42_O + 128], in1=prev, op=MUL)
            TT(out=nxt[:, 0:64], in0=m1[:, 0:64], in1=m1[:, 64:128], op=SUB)
            TT(out=nxt[:, 64:128], in0=m2[:, 0:64], in1=m2[:, 64:128], op=ADD)
            TT(out=nxt, in0=nxt, in1=G2[:, j * 128:(j + 1) * 128], op=ADD)
            warm(nxt)

        # fill h[4k+2] = w^2 (.) h[4k] + G[2k], k=0..3: tiles {1,3,5,7} of
        # the even half from tiles {0,2,4,6} and G tiles {0,2,4,6}
        f2 = workbig.tile([128, 512], F32, tag="f2")
        cmul(f2, w2a, w2b, s3(HSP, 4), s3(GP, 4), 4)
        HOt = HSP.rearrange("p (half k2 two g) -> p half k2 two g",
                            half=2, k2=4, two=2)[:, 0, :, 1, :]
        VC(HOt, g3(f2, 4))
        warm(HSP[:, 7 * 128:8 * 128])

        # odd fill: h[2k+1] = w (.) h[2k] + SS[2k], all k at once
        HE, HO = HSP[:, 0:1024], HSP[:, 1024:2048]
        cmul(HO, w8a, w8b, g3(HE, TK), SSPk[:, 0], TK)
        warm(HO[:, 0:128])

        # repack (p2, k, r, q, b) -> r-split (q, p2, k, b) bf16 for lhsT
        HS2r = big.tile([128, NQ * 128], BF16, tag="hs2r")
        HS2i = big.tile([128, NQ * 128], BF16, tag="hs2i")
        HSPq = HSP.rearrange("p (p2 k r q b) -> p p2 r q k b",
                             p2=2, k=TK, r=2, q=NQ)
        for p2 in range(2):
            for r, dst in ((0, HS2r), (1, HS2i)):
                dv = dst.rearrange("p (q p2 k b) -> p p2 q k b",
                                   q=NQ, p2=2, k=TK)[:, p2]
                VC(dv, HSPq[:, p2, r])

        # ---------------- stage 3b: inter-chunk + intra add + output -------
        y_out = big.tile([128, HL * C], BF16, tag="yout")
        for h in range(HL):
            q, par = h // 2, h % 2
            sl = slice(par * 64, (par + 1) * 64)
            cs = slice(q * 128, (q + 1) * 128)
            ps_y = psy.tile([128, C], F32)
            nc.tensor.matmul(ps_y, HS2r[sl, cs],
                             e2a[sl, q * C:(q + 1) * C],
                             start=True, stop=False)
            nc.tensor.matmul(ps_y, HS2i[sl, cs],
                             e2b[sl, q * C:(q + 1) * C],
                             start=False, stop=True)
            TT(out=y_out[:, h * C:(h + 1) * C],
               in0=ybf[:, h * C:(h + 1) * C], in1=ps_y, op=ADD)
            if h % 2 == 1:
                nc.sync.dma_start(out=y_d[:, (h - 1) * C:(h + 1) * C],
                                  in_=y_out[:, (h - 1) * C:(h + 1) * C])

    nc.compile()
    return nc


def _host_prep(u, Lambda_re, Lambda_im, W_ri, D, log_step):
    """Per-core input dicts; all tables in float64 then cast."""
    u = np.asarray(u, np.float32)
    Lr = np.asarray(Lambda_re, np.float64)
    Li = np.asarray(Lambda_im, np.float64)
    W = np.asarray(W_ri, np.float64)
    Dv = np.asarray(D, np.float64)
    ls = np.asarray(log_step, np.float64)

    step = np.exp(ls)                                    # [H]
    a = step[:, None] * Lr                               # [H,N]
    th = step[:, None] * Li
    lam = a + 1j * th                                    # log z
    z = np.exp(lam)
    w = z ** C
    zL = z ** L
    s = (1.0 - zL) / (1.0 - z)
    Lam = Lr + 1j * Li
    c = (W[..., 0] + 1j * W[..., 1]) / Lam * np.conj(s) / (s * np.conj(s) + EPS)

    e = np.arange(C + 1, dtype=np.float64)
    zp = np.exp(lam[..., None] * e)                      # [H,N,C+1]
    cz = c[..., None] * zp                               # [H,N,C+1]
    K = cz.real.sum(axis=1)                              # [H,C+1]

    # Toeplitz index helpers
    pr = np.arange(128)
    dmat = np.arange(C)[None, :] - pr[:, None]           # [128,C]
    valid = dmat >= 0
    dcl = np.where(valid, dmat, 0)

    idmat = np.eye(128, dtype=np.float64)

    in_maps = []
    for core in range(NCORES):
        hs = slice(core * HL, (core + 1) * HL)

        # u cols (h, J%2, J//2, b): [B,HL,kk,p2,k,128] -> [k][p][h][p2][kk][b]
        uu = np.asarray(u[:, hs]).reshape(B, HL, TK, 2, 2, 128)
        uu_t = np.ascontiguousarray(
            uu.transpose(4, 5, 1, 3, 2, 0)).reshape(2, 128, T * 128)

        # kshift Toeplitz blocks
        Kh = K[hs]                                       # [HL,C+1]
        M = Kh[:, dcl] * valid[None]                     # [HL,128,C]
        M[:, pr, pr] += Dv[hs, None]
        ksh = np.ascontiguousarray(M.transpose(1, 0, 2)).reshape(128, HL * C)

        # stage-1 weights
        zz = zp[hs]                                      # [HL,N,C+1]
        P0 = zz[:, :, 255 - pr]                          # [HL,N,128]
        P1 = zz[:, :, 127 - pr]
        zft0 = np.stack([P0.real, P0.imag], axis=1)      # [HL,2,N,128]
        zft1 = np.stack([P1.real, P1.imag], axis=1)
        zft0 = np.ascontiguousarray(
            zft0.transpose(3, 0, 1, 2)).reshape(128, HL * 128)
        zft1 = np.ascontiguousarray(
            zft1.transpose(3, 0, 1, 2)).reshape(128, HL * 128)

        # stage-3 tables, j = i+1 (cols q*C+i): [q,par,n,j]->rows (par,n)
        czs = cz[hs, :, 1:].reshape(NQ, 2, N, C)
        czt = np.ascontiguousarray(
            czs.transpose(1, 2, 0, 3)).reshape(128, NQ * C)
        e2id = np.concatenate([czt.real, -czt.imag, idmat], axis=1)

        def wpack(x):  # [HL,N] -> rows (par,n), cols (q,b)
            return np.ascontiguousarray(
                np.asarray(x).reshape(HL // 2, 2, N).transpose(1, 2, 0)
            ).reshape(128, HL // 2, 1).repeat(8, axis=2).reshape(128, 64)

        w1, w2, w4 = w[hs], w[hs] ** 2, w[hs] ** 4
        wf = np.concatenate([
            wpack(w2.real), wpack(w2.imag), wpack(w2.imag), wpack(w2.real),
            wpack(w4.real), wpack(w4.imag), wpack(w4.imag), wpack(w4.real),
            wpack(w1.real), wpack(w1.imag), wpack(w1.imag), wpack(w1.real)],
            axis=1)

        m = {
            "ut0": uu_t[0].astype(BF16_NP),
            "ut1": uu_t[1].astype(BF16_NP),
            "zft0": zft0.astype(BF16_NP),
            "zft1": zft1.astype(BF16_NP),
            "ksh": ksh.astype(BF16_NP),
            "e2id": e2id.astype(BF16_NP),
            "wf": wf.astype(np.float32),
        }
        in_maps.append(m)
    return in_maps


def _run(inputs, trace=False):
    if "nc" not in _CACHE:
        _CACHE["nc"] = _build()
    nc = _CACHE["nc"]
    in_maps = _host_prep(**inputs)
    res = run_bass_kernel_spmd(nc, in_maps, list(range(NCORES)), trace=trace)
    parts = []
    for core in range(NCORES):
        # rows (J%2, J//2, b); (kk, p2) reshape restores J = 2*kk + J%2
        ys = res.results[core]["y_s"].astype(np.float32)
        ys = ys.reshape(2, TK, B, HL, C).transpose(2, 3, 1, 0, 4)
        parts.append(ys.reshape(B, HL, L))
    y = np.concatenate(parts, axis=1)                    # [B, H, L]
    return np.ascontiguousarray(y.astype(np.float32)), res


def kernel(**inputs) -> np.ndarray:
    y, _ = _run(inputs, trace=False)
    return y


def kernel_traced(**inputs):
    y, res = _run(inputs, trace=True)
    return y, res


# revision 21
# speedup vs baseline: 1.2100x; 1.1506x over previous
"""DSS layer (S4-style diagonal state space) Trainium2 kernel, v5.

Full inputs:  u [8,128,4096], Lambda_re/im [128,64], W_ri [128,64,2],
              D [128], log_step [128]  ->  y [8,128,4096]

Sharding: H split 8 ways (16 channels/core), all B on every core.

Algorithm (chunked semiseparable scan, C=256, T=16 chunks):
  With z = exp(step*Lambda) (|z|<1), the DSS kernel is
      K[l] = Re sum_n c_n z_n^l,   c = (W/Lambda)*conj(s)/(|s|^2+eps),
      s = (1-z^L)/(1-z)
  and y = causal_conv(K, u) + D*u. Per (b,h) the conv splits into
    intra-chunk:  dense Toeplitz matmul with kshift[j,i] = K[i-j]
                  (+ D on the diagonal), causal-masked
    inter-chunk:  states s[J] = sum_j z^{C-1-j} u[J*C+j]  (matmul),
                  scan h[I] = z^C h[I-1] + s[I-1]          (vector),
                  y_inter = Re sum_n c z^{i+1} h_n[I]      (matmul).

v5: radix-4 scan (pairs -> quads -> 3-step chain -> two fill levels,
35 contiguous vector ops); w tables expanded on-device; DMA priority
order (u/zft first, ksh behind them on the same queue, e2id issue
data-gated behind stage 1); small fp32 matmuls chained to scan outputs
keep the PE p-state up through the scan window; bf16 matmuls + output.
"""
import numpy as np
import ml_dtypes

import concourse.bass as bass
import concourse.bacc as bacc
import concourse.tile as tile
from concourse import mybir
from concourse.bass_utils import run_bass_kernel_spmd

F32 = mybir.dt.float32
BF16 = mybir.dt.bfloat16
BF16_NP = ml_dtypes.bfloat16

B, H, L, N = 8, 128, 4096, 64
NCORES = 8
HL = H // NCORES            # 16 channels per core
C = 256                     # chunk length
T = L // C                  # 16 chunks
TK = T // 2                 # 8 chunk pairs
NQ = HL // 2                # 8 h-pairs
NK = TK - 1                 # 7 pair states feeding the chain
EPS = 1e-7

# e2id col offsets: e2a | e2b | ident
E2A_O, E2B_O, ID_O = 0, NQ * C, 2 * NQ * C
E2ID_W = ID_O + 128
# wf col offsets (fp32): w2p1 | w2p2 | w4p1 | w4p2 | w81 | w82
W2P1_O, W2P2_O, W41_O, W42_O, W81_O, W82_O = 0, 128, 256, 384, 512, 640
WF_W = 768

_CACHE = {}


def _build():
    """Build the SPMD Bass program (identical on all cores)."""
    nc = bacc.Bacc(trn_type="TRN2", target_bir_lowering=False)

    # ---------------- DRAM I/O (per core) ----------------
    # ut_k[p, h*128 + (J%2)*64 + (J//2)*8 + b] = u[b, h, J*256 + k*128 + p]
    ut0_d = nc.dram_tensor("ut0", [128, T * 128], BF16, kind="ExternalInput")
    ut1_d = nc.dram_tensor("ut1", [128, T * 128], BF16, kind="ExternalInput")
    # zft_k[p, h*128+r*64+n] = Re/Im(z^{(255|127)-p})
    zft0_d = nc.dram_tensor("zft0", [128, HL * 128], BF16, kind="ExternalInput")
    zft1_d = nc.dram_tensor("zft1", [128, HL * 128], BF16, kind="ExternalInput")
    # ksh[p, h*C+m] = K[h,m-p] (m>p), K[h,0]+D[h] (m=p), 0 (m<p)
    ksh_d = nc.dram_tensor("ksh", [128, HL * C], BF16, kind="ExternalInput")
    # e2a[par*64+n, q*C+i] = Re(c z^{i+1}), e2b = -Im(c z^{i+1}); ident
    e2id_d = nc.dram_tensor("e2id", [128, E2ID_W], BF16, kind="ExternalInput")
    # scan tables fp32, packed re|im cols (r, q, b): w^2, w^4, w
    wf_d = nc.dram_tensor("wf", [128, WF_W], BF16, kind="ExternalInput")

    # rows = (J%2)*64 + (J//2)*8 + b, cols = h*C+c  (host un-permutes)
    y_d = nc.dram_tensor("y_s", [128, HL * C], BF16, kind="ExternalOutput")

    with tile.TileContext(nc) as tc, \
         tc.tile_pool(name="big", bufs=1) as big, \
         tc.tile_pool(name="work", bufs=2) as work, \
         tc.tile_pool(name="workbig", bufs=1) as workbig, \
         tc.tile_pool(name="ps1", bufs=4, space="PSUM") as ps1, \
         tc.tile_pool(name="psy", bufs=4, space="PSUM") as psy:

        # ---- DMA issue: u/zft gate stage 1, ksh queued behind u ----
        zft0 = big.tile([128, HL * 128], BF16, tag="zft0")
        zft1 = big.tile([128, HL * 128], BF16, tag="zft1")
        ut0 = big.tile([128, T * 128], BF16, tag="ut0")
        ut1 = big.tile([128, T * 128], BF16, tag="ut1")
        ksh = big.tile([128, HL * C], BF16, tag="ksh")
        e2id = big.tile([128, E2ID_W], BF16, tag="e2id")
        wf = big.tile([128, WF_W], BF16, tag="wf")
        nc.scalar.dma_start(out=zft0, in_=zft0_d[:])
        nc.sync.dma_start(out=ut0, in_=ut0_d[:])
        nc.scalar.dma_start(out=zft1, in_=zft1_d[:])
        nc.scalar.dma_start(out=wf, in_=wf_d[:])
        nc.sync.dma_start(out=ut1, in_=ut1_d[:])
        nc.sync.dma_start(out=ksh, in_=ksh_d[:])

        e2a = e2id[:, E2A_O:E2A_O + NQ * C]
        e2b = e2id[:, E2B_O:E2B_O + NQ * C]
        ident = e2id[:, ID_O:ID_O + 128]

        TT = nc.vector.tensor_tensor
        VC = nc.vector.tensor_copy
        MUL = mybir.AluOpType.mult
        ADD = mybir.AluOpType.add
        SUB = mybir.AluOpType.subtract
        uT = [ut0, ut1]
        zft = [zft0, zft1]

        # packed state tiles: cols (J%2, J//2, r, q, b), r = re|im
        SSP = big.tile([128, T * 128], BF16, tag="ssp")
        HSP = big.tile([128, T * 128], BF16, tag="hsp")
        nc.vector.memset(HSP[:, 0:128], 0.0)

        # expand w tables over chunk-pair groups (vector, idle window)
        w2a = big.tile([128, 512], BF16, tag="w2a")
        w2b = big.tile([128, 512], BF16, tag="w2b")
        w8a = big.tile([128, TK * 128], BF16, tag="w8a")
        w8b = big.tile([128, TK * 128], BF16, tag="w8b")
        for dst, off, reps, eng in (
                (w8a, W81_O, 3, nc.gpsimd), (w8b, W82_O, 3, nc.gpsimd),
                (w2a, W2P1_O, 2, nc.gpsimd), (w2b, W2P2_O, 2, nc.gpsimd)):
            eng.tensor_copy(dst[:, 0:128], wf[:, off:off + 128])
            for i in range(reps):
                n = 128 << i
                eng.tensor_copy(dst[:, n:2 * n], dst[:, 0:n])

        # ---------------- stage 1: chunk states ----------------------------
        SSPv = SSP.rearrange("p (p2 k r q b) -> p p2 k r q b",
                             p2=2, k=TK, r=2, q=NQ)
        for h in range(HL):
            q, par = h // 2, h % 2
            ps_s = ps1.tile([128, 128], F32, tag='ps1t')
            for k in range(2):
                nc.tensor.matmul(ps_s, zft[k][:, h * 128:(h + 1) * 128],
                                 uT[k][:, h * 128:(h + 1) * 128],
                                 start=(k == 0), stop=(k == 1))
            sl = slice(par * 64, (par + 1) * 64)
            srcv = ps_s.rearrange("p (p2 k b) -> p p2 k b", p2=2, k=TK)
            VC(SSPv[sl, :, :, 0, q, :], srcv[0:64])
            VC(SSPv[sl, :, :, 1, q, :], srcv[64:128])
            if h == 0:
                # WAW dep: write into e2id tile from SSP so the e2id DMA
                # cannot start before stage 1 is running (bandwidth gate)
                nc.gpsimd.tensor_copy(e2id[0:1, 0:4], SSP[0:1, 0:4])
                nc.gpsimd.dma_start(out=e2id, in_=e2id_d[:])

        # ---------------- stage 3a: intra-chunk (scan-independent) ---------
        ybf = big.tile([128, HL * C], F32, tag="ybf")
        for h in range(HL):
            ps_y = psy.tile([128, C], F32)
            nc.tensor.matmul(ps_y, ut0[:, h * 128:(h + 1) * 128],
                             ksh[:, h * C:h * C + C], start=True, stop=False)
            nc.tensor.matmul(ps_y[:, 128:C], ut1[:, h * 128:(h + 1) * 128],
                             ksh[:, h * C:h * C + 128], start=False, stop=True)
            nc.scalar.copy(ybf[:, h * C:(h + 1) * C], ps_y)

        # ---------------- stage 2: radix-4 scan (vector, fp32) -------------
        # warm-keeper: tiny fp32 matmul chained to a freshly written tile
        def warm(src128):
            psd = ps1.tile([128, 128], F32, tag='ps1t')
            nc.tensor.matmul(psd, wf[:, 0:128], src128,
                             start=True, stop=True)

        def cmul5(dst, dstv, wa, wb, src, add_t, width):
            """dst = w (.) src + add_t; dstv = [p, k, r, 64] view of dst."""
            m1 = workbig.tile([128, width], F32, tag=f"m1_{width}")
            TT(out=m1, in0=wa, in1=src, op=MUL)
            m2 = workbig.tile([128, width], F32, tag=f"m2_{width}")
            TT(out=m2, in0=wb, in1=src, op=MUL)
            m1v = m1.rearrange("p (k r x) -> p k r x", r=2, x=64)
            m2v = m2.rearrange("p (k r x) -> p k r x", r=2, x=64)
            TT(out=dstv[:, :, 0], in0=m1v[:, :, 0], in1=m1v[:, :, 1], op=SUB)
            TT(out=dstv[:, :, 1], in0=m2v[:, :, 0], in1=m2v[:, :, 1], op=ADD)
            TT(out=dst, in0=dst, in1=add_t, op=ADD)

        def g3(t, nk):          # [p, nk, 128] view of contiguous 128-groups
            return t.rearrange("p (k g) -> p k g", k=nk) if nk > 1 else t

        def s3(t, nk, g0=0):    # [p, nk, 128] view at stride 256 from g0
            a0 = t[:, g0 * 128:g0 * 128 + 128]
            return bass.AP(tensor=a0.tensor, offset=a0.offset,
                           ap=[list(a0.ap[0]), [256, nk], [1, 128]])

        def cmul(dst, wa, wb, src, add_t, nk):
            """dst = w (.) src + add_t over nk packed (r,q,b) 128-groups.

            dst must be a contiguous [128, nk*128] region; wa/wb contiguous
            weight regions; src/add_t any [p, nk, 128]-shaped views.
            """
            m1 = workbig.tile([128, nk * 128], BF16, tag=f"m1_{nk}")
            TT(out=g3(m1, nk), in0=g3(wa, nk), in1=src, op=MUL)
            m2 = workbig.tile([128, nk * 128], BF16, tag=f"m2_{nk}")
            TT(out=g3(m2, nk), in0=g3(wb, nk), in1=src, op=MUL)
            m1v = m1.rearrange("p (k r x) -> p k r x", r=2, x=64)
            m2v = m2.rearrange("p (k r x) -> p k r x", r=2, x=64)
            dv = dst.rearrange("p (k r x) -> p k r x", r=2, x=64)
            TT(out=dv[:, :, 0], in0=m1v[:, :, 0], in1=m1v[:, :, 1], op=SUB)
            TT(out=dv[:, :, 1], in0=m2v[:, :, 0], in1=m2v[:, :, 1], op=ADD)
            TT(out=g3(dst, nk), in0=g3(dst, nk), in1=add_t, op=ADD)

        SSPk = SSP.rearrange("p (p2 k g) -> p p2 k g", p2=2, k=TK)
        # G[k] = w (.) SS[2k] + SS[2k+1], k=0..6
        GP = big.tile([128, NK * 128], BF16, tag="gp")
        cmul(GP, w8a[:, 0:NK * 128], w8b[:, 0:NK * 128],
             SSPk[:, 0, 0:NK, :], SSPk[:, 1, 0:NK, :], NK)
        warm(GP[:, 0:128])

        # G2[k] = w^2 (.) G[2k] + G[2k+1], k=0..2
        G2 = big.tile([128, 3 * 128], BF16, tag="g2")
        cmul(G2, w2a[:, 0:384], w2b[:, 0:384],
             s3(GP, 3), s3(GP, 3, g0=1), 3)
        warm(G2[:, 0:128])

        # chain over h[4k]: 3 steps, h[4k+4] = w^4 (.) h[4k] + G2[k]
        for j in range(3):
            prev = HSP[:, (2 * j) * 128:(2 * j + 1) * 128]
            nxt = HSP[:, (2 * j + 2) * 128:(2 * j + 3) * 128]
            m1 = work.tile([128, 128], BF16, tag="c4m1")
            TT(out=m1, in0=wf[:, W41_O:W41_O + 128], in1=prev, op=MUL)
            m2 = work.tile([128, 128], BF16, tag="c4m2")
            TT(out=m2, in0=wf[:, W42_O:W42_O + 128], in1=prev, op=MUL)
            TT(out=nxt[:, 0:64], in0=m1[:, 0:64], in1=m1[:, 64:128], op=SUB)
            TT(out=nxt[:, 64:128], in0=m2[:, 0:64], in1=m2[:, 64:128], op=ADD)
            TT(out=nxt, in0=nxt, in1=G2[:, j * 128:(j + 1) * 128], op=ADD)
            warm(nxt)

        # fill h[4k+2] = w^2 (.) h[4k] + G[2k], k=0..3: tiles {1,3,5,7} of
        # the even half from tiles {0,2,4,6} and G tiles {0,2,4,6}
        f2 = workbig.tile([128, 512], BF16, tag="f2")
        cmul(f2, w2a, w2b, s3(HSP, 4), s3(GP, 4), 4)
        HOt = HSP.rearrange("p (half k2 two g) -> p half k2 two g",
                            half=2, k2=4, two=2)[:, 0, :, 1, :]
        VC(HOt, g3(f2, 4))
        warm(HSP[:, 7 * 128:8 * 128])

        # odd fill: h[2k+1] = w (.) h[2k] + SS[2k], all k at once
        HE, HO = HSP[:, 0:1024], HSP[:, 1024:2048]
        cmul(HO, w8a, w8b, g3(HE, TK), SSPk[:, 0], TK)
        warm(HO[:, 0:128])

        # repack (p2, k, r, q, b) -> r-split (q, p2, k, b) bf16 for lhsT
        HS2r = big.tile([128, NQ * 128], BF16, tag="hs2r")
        HS2i = big.tile([128, NQ * 128], BF16, tag="hs2i")
        HSPq = HSP.rearrange("p (p2 k r q b) -> p p2 r q k b",
                             p2=2, k=TK, r=2, q=NQ)
        for p2 in range(2):
            for r, dst in ((0, HS2r), (1, HS2i)):
                dv = dst.rearrange("p (q p2 k b) -> p p2 q k b",
                                   q=NQ, p2=2, k=TK)[:, p2]
                VC(dv, HSPq[:, p2, r])

        # ---------------- stage 3b: inter-chunk + intra add + output -------
        y_out = big.tile([128, HL * C], BF16, tag="yout")
        for h in range(HL):
            q, par = h // 2, h % 2
            sl = slice(par * 64, (par + 1) * 64)
            cs = slice(q * 128, (q + 1) * 128)
            ps_y = psy.tile([128, C], F32)
            nc.tensor.matmul(ps_y, HS2r[sl, cs],
                             e2a[sl, q * C:(q + 1) * C],
                             start=True, stop=False)
            nc.tensor.matmul(ps_y, HS2i[sl, cs],
                             e2b[sl, q * C:(q + 1) * C],
                             start=False, stop=True)
            TT(out=y_out[:, h * C:(h + 1) * C],
               in0=ybf[:, h * C:(h + 1) * C], in1=ps_y, op=ADD)
            if h % 2 == 1:
                nc.sync.dma_start(out=y_d[:, (h - 1) * C:(h + 1) * C],
                                  in_=y_out[:, (h - 1) * C:(h + 1) * C])

    nc.compile()
    return nc


def _host_prep(u, Lambda_re, Lambda_im, W_ri, D, log_step):
    """Per-core input dicts; all tables in float64 then cast."""
    u = np.asarray(u, np.float32)
    Lr = np.asarray(Lambda_re, np.float64)
    Li = np.asarray(Lambda_im, np.float64)
    W = np.asarray(W_ri, np.float64)
    Dv = np.asarray(D, np.float64)
    ls = np.asarray(log_step, np.float64)

    step = np.exp(ls)                                    # [H]
    a = step[:, None] * Lr                               # [H,N]
    th = step[:, None] * Li
    lam = a + 1j * th                                    # log z
    z = np.exp(lam)
    w = z ** C
    zL = z ** L
    s = (1.0 - zL) / (1.0 - z)
    Lam = Lr + 1j * Li
    c = (W[..., 0] + 1j * W[..., 1]) / Lam * np.conj(s) / (s * np.conj(s) + EPS)

    e = np.arange(C + 1, dtype=np.float64)
    zp = np.exp(lam[..., None] * e)                      # [H,N,C+1]
    cz = c[..., None] * zp                               # [H,N,C+1]
    K = cz.real.sum(axis=1)                              # [H,C+1]

    # Toeplitz index helpers
    pr = np.arange(128)
    dmat = np.arange(C)[None, :] - pr[:, None]           # [128,C]
    valid = dmat >= 0
    dcl = np.where(valid, dmat, 0)

    idmat = np.eye(128, dtype=np.float64)

    in_maps = []
    for core in range(NCORES):
        hs = slice(core * HL, (core + 1) * HL)

        # u cols (h, J%2, J//2, b): [B,HL,kk,p2,k,128] -> [k][p][h][p2][kk][b]
        uu = np.asarray(u[:, hs]).reshape(B, HL, TK, 2, 2, 128)
        uu_t = np.ascontiguousarray(
            uu.transpose(4, 5, 1, 3, 2, 0)).reshape(2, 128, T * 128)

        # kshift Toeplitz blocks
        Kh = K[hs]                                       # [HL,C+1]
        M = Kh[:, dcl] * valid[None]                     # [HL,128,C]
        M[:, pr, pr] += Dv[hs, None]
        ksh = np.ascontiguousarray(M.transpose(1, 0, 2)).reshape(128, HL * C)

        # stage-1 weights
        zz = zp[hs]                                      # [HL,N,C+1]
        P0 = zz[:, :, 255 - pr]                          # [HL,N,128]
        P1 = zz[:, :, 127 - pr]
        zft0 = np.stack([P0.real, P0.imag], axis=1)      # [HL,2,N,128]
        zft1 = np.stack([P1.real, P1.imag], axis=1)
        zft0 = np.ascontiguousarray(
            zft0.transpose(3, 0, 1, 2)).reshape(128, HL * 128)
        zft1 = np.ascontiguousarray(
            zft1.transpose(3, 0, 1, 2)).reshape(128, HL * 128)

        # stage-3 tables, j = i+1 (cols q*C+i): [q,par,n,j]->rows (par,n)
        czs = cz[hs, :, 1:].reshape(NQ, 2, N, C)
        czt = np.ascontiguousarray(
            czs.transpose(1, 2, 0, 3)).reshape(128, NQ * C)
        e2id = np.concatenate([czt.real, -czt.imag, idmat], axis=1)

        def wpack(x):  # [HL,N] -> rows (par,n), cols (q,b)
            return np.ascontiguousarray(
                np.asarray(x).reshape(HL // 2, 2, N).transpose(1, 2, 0)
            ).reshape(128, HL // 2, 1).repeat(8, axis=2).reshape(128, 64)

        w1, w2, w4 = w[hs], w[hs] ** 2, w[hs] ** 4
        wf = np.concatenate([
            wpack(w2.real), wpack(w2.imag), wpack(w2.imag), wpack(w2.real),
            wpack(w4.real), wpack(w4.imag), wpack(w4.imag), wpack(w4.real),
            wpack(w1.real), wpack(w1.imag), wpack(w1.imag), wpack(w1.real)],
            axis=1)

        m = {
            "ut0": uu_t[0].astype(BF16_NP),
            "ut1": uu_t[1].astype(BF16_NP),
            "zft0": zft0.astype(BF16_NP),
            "zft1": zft1.astype(BF16_NP),
            "ksh": ksh.astype(BF16_NP),
            "e2id": e2id.astype(BF16_NP),
            "wf": wf.astype(BF16_NP),
        }
        in_maps.append(m)
    return in_maps


def _run(inputs, trace=False):
    if "nc" not in _CACHE:
        _CACHE["nc"] = _build()
    nc = _CACHE["nc"]
    in_maps = _host_prep(**inputs)
    res = run_bass_kernel_spmd(nc, in_maps, list(range(NCORES)), trace=trace)
    parts = []
    for core in range(NCORES):
        # rows (J%2, J//2, b); (kk, p2) reshape restores J = 2*kk + J%2
        ys = res.results[core]["y_s"].astype(np.float32)
        ys = ys.reshape(2, TK, B, HL, C).transpose(2, 3, 1, 0, 4)
        parts.append(ys.reshape(B, HL, L))
    y = np.concatenate(parts, axis=1)                    # [B, H, L]
    return np.ascontiguousarray(y.astype(np.float32)), res


def kernel(**inputs) -> np.ndarray:
    y, _ = _run(inputs, trace=False)
    return y


def kernel_traced(**inputs):
    y, res = _run(inputs, trace=True)
    return y, res
